# revision 1
# baseline (speedup 1.0000x reference)
"""BinarizeLinear inference kernel for 8 Trainium2 NeuronCores.

Computes out = sign(input) @ sign(weight) + bias with sign(x) = +1 if x > 0
else -1, for input [8192, 4096] fp32, weight [4096, 4096] fp32, bias [4096].

Strategy: 4x2 (rows x cols) sharding across 8 cores — the DMA-optimal split
(per core: 32 MB x + 32 MB w in, 16 MB out). Each core computes a
[2048, 2048] output shard from x rows [2048, 4096] and w cols [4096, 2048].
On-chip per core:
  - binarize w and x to fp8e4 (+-1 is exact in fp8) with the ACT Sign LUT;
    x arrives host-pre-permuted per m-tile in [ki, j, m] layout so the sign
    lands directly in the lhsT layout (no on-chip transpose at all)
  - main GEMM in fp8 DoubleRow perf mode (256-deep contraction per matmul,
    ~213 ns per [256 x 128] x [256 x 512] matmul warm), accumulating exactly
    in fp32 PSUM (all partial sums are integers <= 4096)
  - bias add fused into the PSUM->SBUF copy on the DVE
  - w arrives n-block-major via host-side pre-permute (contiguous 1 MiB quad
    DMAs); DMA issue order, ACT sign order and the in-order PE instruction
    stream are co-scheduled so the PE ramps at ~25 us and then runs dense
  - a short identity-matmul warmup bridges the w-block-0 DMA gate and keeps
    the PE HAM clock gate at 2.4 GHz

Measured on 8 axon TRN2 cores: 308 us HW exec (up to ~10% more when the
chip is power-throttled from repeated benchmarking),
bit-exact vs the fp32 reference (relative error 0.0). PE busy ~253 us,
DVE ~45 us; the dense phase runs at the fp8 DoubleRow streaming floor
(~213-230 ns per matmul), the ramp is input-bandwidth-bound, and ~14 us is
Tile's fixed drain/barrier tail.
"""

import numpy as np

M_FULL, K_FULL, N_FULL = 8192, 4096, 4096
R_SHARDS, C_SHARDS = 4, 2
N_CORES = R_SHARDS * C_SHARDS
M_SHARD = M_FULL // R_SHARDS  # 2048
N_SHARD = N_FULL // C_SHARDS  # 2048
P = 128
NT = 512  # moving free dim per matmul (one PSUM bank of fp32)


def build_nc(M=M_SHARD, K=K_FULL, N=N_SHARD, use_double_row=True, mblk_size=8):
    """Build the single-core Bass program (SPMD: same program on all cores).

    Loop structure (v2):
      - w is loaded n-block-major in [128, 4, 512] "quad" tiles (4 k-chunks)
        on the SP HWDGE queue, so the first output-block matmuls can start
        after ~1/NB of the w stream has landed.
      - x loads ride the ACT HWDGE queue and out stores the gpsimd SWDGE
        queue, so the three streams round-robin on the SDMA engines instead
        of serializing behind one another.
      - m-tiles are processed in blocks of `mblk_size`; within a block the
        output-column blocks (b) are the outer loop so PE stays dense while
        later w blocks stream in.
    """
    import concourse.mybir as mybir
    from concourse import bacc
    from concourse.masks import make_identity
    from concourse.tile import TileContext

    fp32 = mybir.dt.float32
    fp8 = mybir.dt.float8e4

    QUAD = 4  # k-chunks per w tile
    assert M % P == 0 and K % (P * QUAD) == 0 and N % NT == 0
    KSUB = K // P  # number of 128-deep k-chunks
    NQ = KSUB // QUAD  # w quad tiles per n-block
    NB = N // NT  # output column blocks
    MT = M // P  # m-tiles
    mblk_size = min(mblk_size, MT)
    assert MT % mblk_size == 0
    if use_double_row:
        assert KSUB % 2 == 0

    nc = bacc.Bacc()
    # x is pre-permuted on the host per m-tile: x_dev[mi, ki, j, m] =
    # x[mi*P + m, j*P + ki] — each m-tile is one contiguous 2 MiB DMA that
    # lands directly in the [Ki, ksub, m] lhsT layout (no on-chip transpose).
    x = nc.declare_dram_parameter("x", [M // P, P, KSUB, P], fp32, isOutput=False)
    # w is pre-permuted on the host into quad-major layout:
    # w_dev[b*NQ+q, ki, j, n] = w[(q*QUAD+j)*P + ki, b*NT + n], so each
    # [P, QUAD, NT] quad tile is one fully contiguous 1 MiB DMA.
    w = nc.declare_dram_parameter("w", [NB * NQ, P, QUAD, NT], fp32, isOutput=False)
    # bias comes pre-replicated across the 128 partitions from the host
    b = nc.declare_dram_parameter("b", [P, N], fp32, isOutput=False)
    out = nc.declare_dram_parameter("out", [M, N], fp32, isOutput=True)

    with TileContext(nc) as tc:
        with (
            tc.tile_pool(name="const", bufs=1) as cpool,
            tc.tile_pool(name="win", bufs=4) as winp,
            tc.tile_pool(name="wq", bufs=1) as wqp,
            tc.tile_pool(name="xin", bufs=2) as xinp,
            tc.tile_pool(name="xbt", bufs=mblk_size + 3) as xbtp,
            tc.tile_pool(name="ost", bufs=4) as ostp,
            tc.tile_pool(name="mpsum", bufs=4, space="PSUM") as mpp,
            tc.tile_pool(name="wpsum", bufs=1, space="PSUM") as wpp,
        ):
            ident32 = cpool.tile([P, P], fp32)
            make_identity(nc, ident32)
            ident = cpool.tile([P, P], fp8)
            nc.vector.tensor_copy(ident, ident32)

            bias_rep = cpool.tile([P, N], fp32)
            nc.scalar.dma_start(bias_rep, b[:, :])

            # Binarized weight in n-block-major quad tiles: wq[b*NQ+q] holds
            # k-chunks 4q..4q+3 for output columns [b*NT, (b+1)*NT).
            wq = [None] * (NB * NQ)

            def emit_w_quad(bi, q):
                w_in = winp.tile(
                    [P, QUAD, NT], fp32, tag="w_in", name=f"w_in_{bi}_{q}"
                )
                nc.sync.dma_start(w_in, w[bi * NQ + q])
                wt = wqp.tile(
                    [P, QUAD, NT], fp8, tag=f"wq{bi}_{q}", name=f"wq_{bi}_{q}"
                )
                nc.scalar.sign(wt, w_in)
                wq[bi * NQ + q] = wt

            xbts_all = [None] * MT

            def emit_x(mi):
                x_in = xinp.tile([P, KSUB, P], fp32, tag="x_in", name=f"x_in_{mi}")
                nc.sync.dma_start(x_in, x[mi])
                xbT = xbtp.tile([P, KSUB, P], fp8, tag="xbT", name=f"xbT_{mi}")
                nc.scalar.sign(xbT, x_in)
                xbts_all[mi] = xbT

            # DMA / ACT issue order, matched to when the in-order PE needs
            # each piece: x0 first (first transposes), then all of w block 0
            # (gates the first matmul wave), x1, then x2..x7 interleaved with
            # w block 1, w block 2, x8-x9, w block 3, x10..x15.
            first_xs = list(range(min(mblk_size, MT)))
            if NB >= 2:
                emit_x(0)
                for q in range(NQ):
                    emit_w_quad(0, q)
                for mi in first_xs[1:2]:
                    emit_x(mi)
                rem = first_xs[2:]
                qi = 0
                for i in range(0, len(rem), 2):
                    for mi in rem[i : i + 2]:
                        emit_x(mi)
                    take = min(2, NQ - qi) if rem else NQ
                    for _ in range(take):
                        emit_w_quad(1, qi)
                        qi += 1
                while qi < NQ:
                    emit_w_quad(1, qi)
                    qi += 1
                for bi in range(2, NB):
                    for q in range(NQ):
                        emit_w_quad(bi, q)
                for mi in range(len(first_xs), MT):
                    emit_x(mi)
            else:
                for q in range(NQ):
                    emit_w_quad(0, q)
                for mi in range(MT):
                    emit_x(mi)

            # PE warmup: ~250 back-to-back small matmuls bridge the w-block-0
            # DMA wait and move the HAM clock gate to 2.4 GHz before the real
            # matmul stream starts.
            if NB >= 2:
                warm = wpp.tile([P, P], fp32, tag="warm", name="warm")
                for _ in range(150):
                    nc.tensor.matmul(warm, ident, ident, start=True, stop=True)

            def mm_group(mp, xbT, bi):
                if use_double_row:
                    for j2 in range(KSUB // 2):
                        q, r = divmod(j2, 2)
                        nc.tensor.matmul(
                            mp,
                            xbT[:, 2 * j2 : 2 * j2 + 2, :],
                            wq[bi * NQ + q][:, 2 * r : 2 * r + 2, :],
                            start=(j2 == 0),
                            stop=(j2 == KSUB // 2 - 1),
                            perf_mode=mybir.MatmulPerfMode.DoubleRow,
                        )
                else:
                    for j in range(KSUB):
                        q, r = divmod(j, QUAD)
                        nc.tensor.matmul(
                            mp,
                            xbT[:, j, :],
                            wq[bi * NQ + q][:, r, :],
                            start=(j == 0),
                            stop=(j == KSUB - 1),
                        )

            def emit_group(xbT, mi, bi, split=False):
                bsl = slice(bi * NT, (bi + 1) * NT)
                if split and use_double_row and KSUB % 4 == 0:
                    # Two half-depth accumulation groups so the first matmuls
                    # gate on only the first half of the w block's quads.
                    # Halves are exact integers: sum them first, add bias
                    # last — identical rounding to the single-group path.
                    half = KSUB // 4
                    mps = []
                    for h in range(2):
                        mp = mpp.tile(
                            [P, NT], fp32, tag="mp", name=f"mp_{mi}_{bi}_{h}"
                        )
                        for j2 in range(h * half, (h + 1) * half):
                            q, r = divmod(j2, 2)
                            nc.tensor.matmul(
                                mp,
                                xbT[:, 2 * j2 : 2 * j2 + 2, :],
                                wq[bi * NQ + q][:, 2 * r : 2 * r + 2, :],
                                start=(j2 == h * half),
                                stop=(j2 == (h + 1) * half - 1),
                                perf_mode=mybir.MatmulPerfMode.DoubleRow,
                            )
                        mps.append(mp)
                    ost = ostp.tile([P, NT], fp32, tag="ost", name=f"ost_{mi}_{bi}")
                    nc.vector.tensor_tensor(
                        ost, mps[0], mps[1], op=mybir.AluOpType.add
                    )
                    nc.vector.tensor_tensor(
                        ost, ost, bias_rep[:, bsl], op=mybir.AluOpType.add
                    )
                else:
                    mp = mpp.tile([P, NT], fp32, tag="mp", name=f"mp_{mi}_{bi}")
                    mm_group(mp, xbT, bi)
                    ost = ostp.tile([P, NT], fp32, tag="ost", name=f"ost_{mi}_{bi}")
                    nc.vector.tensor_tensor(
                        ost, mp, bias_rep[:, bsl], op=mybir.AluOpType.add
                    )
                nc.gpsimd.dma_start(out[mi * P : (mi + 1) * P, bsl], ost)

            # PE order: per m-block, fused transpose + first-block group per
            # m-tile (so PE starts as soon as x0 and w block 0 land), then
            # the remaining column-block waves.
            for mb in range(MT // mblk_size):
                blk = list(range(mb * mblk_size, (mb + 1) * mblk_size))
                xbts = {}
                # note: interleaving the transpose matmuls inside the b0 DR
                # accumulation group (emit_fused_t_g0) measured SLOWER (+8 us
                # PE busy) — it breaks the DR stream's LDWEIGHTS pipelining.
                for mi in blk:
                    xbts[mi] = xbts_all[mi]
                    emit_group(xbts[mi], mi, 0)
                for bi in range(1, NB):
                    for mi in blk:
                        emit_group(xbts[mi], mi, bi)
    nc.finalize()
    return nc


def permute_x(x_rows, K=K_FULL):
    """[M, K] -> [M//P, P, KSUB, P] per-m-tile [ki, j, m] lhsT layout."""
    M = x_rows.shape[0]
    ksub = K // P
    r = x_rows.reshape(M // P, P, ksub, P)  # [mi, m, j, ki]
    return np.ascontiguousarray(r.transpose(0, 3, 2, 1))


def permute_w(w_col, K=K_FULL, N=N_SHARD, quad=4, nt=NT):
    """[K, N] -> [NB*NQ, P, QUAD, NT] quad-major device layout."""
    nq = K // (P * quad)
    nb = N // nt
    r = w_col.reshape(nq, quad, P, nb, nt)
    return np.ascontiguousarray(
        r.transpose(3, 0, 2, 1, 4).reshape(nb * nq, P, quad, nt)
    )


def _make_in_maps(input, weight, bias):
    x_np = np.asarray(input, dtype=np.float32)
    w_np = np.asarray(weight, dtype=np.float32)
    b_np = np.asarray(bias, dtype=np.float32).reshape(1, -1)
    x_rows = [
        permute_x(x_np[r * M_SHARD : (r + 1) * M_SHARD, :])
        for r in range(R_SHARDS)
    ]
    w_cols = [
        permute_w(w_np[:, c * N_SHARD : (c + 1) * N_SHARD])
        for c in range(C_SHARDS)
    ]
    b_cols = [
        np.ascontiguousarray(
            np.broadcast_to(b_np[:, c * N_SHARD : (c + 1) * N_SHARD], (P, N_SHARD))
        )
        for c in range(C_SHARDS)
    ]
    in_maps = []
    for core in range(N_CORES):
        r, c = divmod(core, C_SHARDS)
        in_maps.append(
            {
                "x": x_rows[r],
                "w": w_cols[c],
                "b": b_cols[c],
            }
        )
    return in_maps


def _assemble(results):
    out = np.empty((M_FULL, N_FULL), dtype=np.float32)
    for core in range(N_CORES):
        r, c = divmod(core, C_SHARDS)
        out[r * M_SHARD : (r + 1) * M_SHARD, c * N_SHARD : (c + 1) * N_SHARD] = (
            results[core]["out"]
        )
    return out


def run(input, weight, bias, trace=False, trace_cores=None):
    """Run on 8 NeuronCores; returns (output, BassKernelResults)."""
    from concourse.bass_utils import run_bass_kernel_spmd

    nc = build_nc()
    in_maps = _make_in_maps(input, weight, bias)
    res = run_bass_kernel_spmd(
        nc, in_maps, list(range(N_CORES)), trace=trace, trace_cores=trace_cores
    )
    return _assemble(res.results), res


def kernel(input, weight, bias):
    out, _ = run(input, weight, bias)
    return out



# revision 4
# speedup vs baseline: 1.2823x; 1.2823x over previous
"""BinarizeLinear inference kernel for 8 Trainium2 NeuronCores.

Computes out = sign(input) @ sign(weight) + bias with sign(x) = +1 if x > 0
else -1, for input [8192, 4096] fp32, weight [4096, 4096] fp32, bias [4096].

Strategy: 4x2 (rows x cols) sharding across 8 cores — the DMA-optimal split.
Each core computes a [2048, 2048] output shard from x rows [2048, 4096] and
w cols [4096, 2048].

v3 design (vs the 342 us v2 baseline):
  - inputs staged to the device as bf16 (sign-exact for randn data): input
    DMA halves to 32 MiB/core, so the in-stream lands in ~80 us and the
    ramp is no longer DMA-starved
  - x is binarized on the DVE as x_hat = 2*(x>0) in {0,2} fp8 (one
    tensor_scalar op); the identity sum(sign x * sign w) =
    sum(x_hat * sign w) - colsum(sign w) is folded into a host-adjusted
    bias b' = bias - colsum(sign w), so ACT only signs w (64 us) and is
    never the ramp gate
  - main GEMM unchanged: fp8 DoubleRow, 256-deep per matmul, fp32 PSUM
    (exact: partial sums are even integers <= 8192), ~213 ns per
    [256 x 128] x [256 x 512] matmul warm -> 218 us PE floor per core
  - output written as bf16 (exact to ~1e-3 of max; tolerance is 2e-2),
    halving the out stream to 8 MiB
  - DMA / ACT / DVE streams hand-ordered so each w block and x tile is
    resident just before the in-order PE stream reaches it
"""

import numpy as np
import ml_dtypes

M_FULL, K_FULL, N_FULL = 8192, 4096, 4096
R_SHARDS, C_SHARDS = 4, 2
N_CORES = R_SHARDS * C_SHARDS
M_SHARD = M_FULL // R_SHARDS  # 2048
N_SHARD = N_FULL // C_SHARDS  # 2048
P = 128
NT = 512  # moving free dim per matmul (one PSUM bank of fp32)

BF16 = ml_dtypes.bfloat16


def build_nc(M=M_SHARD, K=K_FULL, N=N_SHARD, mblk_size=8, n_warmup=120):
    """Build the single-core Bass program (SPMD: same program on all cores)."""
    import concourse.mybir as mybir
    from concourse import bacc
    from concourse.masks import make_identity
    from concourse.tile import TileContext

    fp32 = mybir.dt.float32
    bf16 = mybir.dt.bfloat16
    fp8 = mybir.dt.float8e4

    QUAD = 4  # k-chunks per w tile
    assert M % P == 0 and K % (P * QUAD) == 0 and N % NT == 0
    KSUB = K // P  # 32 k-chunks of 128
    NQ = KSUB // QUAD  # 8 w quad tiles per n-block
    NB = N // NT  # 4 output column blocks
    MT = M // P  # 16 m-tiles
    mblk_size = min(mblk_size, MT)
    assert MT % mblk_size == 0 and KSUB % 2 == 0

    nc = bacc.Bacc()
    # x pre-permuted on host per m-tile: x_dev[mi, ki, j, m] = x[mi*P+m, j*P+ki]
    # (bf16) — each m-tile is one contiguous 1 MiB DMA in lhsT layout.
    x = nc.declare_dram_parameter("x", [MT, P, KSUB, P], bf16, isOutput=False)
    # w pre-permuted on host into quad-major layout (bf16):
    # w_dev[b*NQ+q, ki, j, n] = w[(q*QUAD+j)*P + ki, b*NT + n] — 0.5 MiB quads.
    w = nc.declare_dram_parameter("w", [NB * NQ, P, QUAD, NT], bf16, isOutput=False)
    # b' = bias - colsum(sign w), pre-replicated across partitions (fp32).
    b = nc.declare_dram_parameter("b", [P, N], fp32, isOutput=False)
    out = nc.declare_dram_parameter("out", [M, N], bf16, isOutput=True)

    with TileContext(nc) as tc:
        with (
            tc.tile_pool(name="const", bufs=1) as cpool,
            tc.tile_pool(name="win", bufs=6) as winp,
            tc.tile_pool(name="wq", bufs=1) as wqp,
            tc.tile_pool(name="xin", bufs=4) as xinp,
            tc.tile_pool(name="xbt", bufs=1) as xbtp,
            tc.tile_pool(name="ost", bufs=6) as ostp,
            tc.tile_pool(name="mpsum", bufs=6, space="PSUM") as mpp,
            tc.tile_pool(name="wpsum", bufs=1, space="PSUM") as wpp,
        ):
            ident32 = cpool.tile([P, P], fp32)
            make_identity(nc, ident32)
            ident = cpool.tile([P, P], fp8)
            nc.vector.tensor_copy(ident, ident32)

            bias_rep = cpool.tile([P, N], fp32)
            nc.scalar.dma_start(bias_rep, b[:, :])

            wq = [None] * (NB * NQ)

            def emit_w_quad(bi, q):
                w_in = winp.tile([P, QUAD, NT], bf16, tag="w_in", name=f"w_in_{bi}_{q}")
                nc.sync.dma_start(w_in, w[bi * NQ + q])
                wt = wqp.tile([P, QUAD, NT], fp8, tag=f"wq{bi}_{q}", name=f"wq_{bi}_{q}")
                nc.scalar.sign(wt, w_in)
                wq[bi * NQ + q] = wt

            xbts = [None] * MT
            x_binned = [False] * MT

            def emit_x_dma(mi):
                x_in = xinp.tile([P, KSUB, P], bf16, tag="x_in", name=f"x_in_{mi}")
                nc.sync.dma_start(x_in, x[mi])
                xbts[mi] = x_in  # raw tile until binarized

            def emit_x_bin(mi):
                if x_binned[mi]:
                    return
                x_binned[mi] = True
                x_in = xbts[mi]
                xbT = xbtp.tile([P, KSUB, P], fp8, tag=f"xbT{mi}", name=f"xbT_{mi}")
                # x_hat = 2 * (x > 0) in {0, 2}; exact in fp8.
                nc.vector.tensor_scalar(
                    xbT, x_in, 0.0, 2.0, mybir.AluOpType.is_gt, mybir.AluOpType.mult
                )
                xbts[mi] = xbT

            # ---- DMA issue order (single sync HWDGE FIFO; ~0.4 MiB/us) ----
            # x0,x1 | b0 quads | x2,x3 | b1 q0-3 | x4..x7 | b1 q4-7 | b2 |
            # x8-11 | b3 | x12-15 — each piece lands just before the PE
            # stream needs it; ACT (w signs only) follows the same order.
            emit_x_dma(0)
            emit_x_dma(1)
            emit_x_bin(0)
            emit_x_bin(1)
            for q in range(NQ):
                emit_w_quad(0, q)
            emit_x_dma(2)
            emit_x_dma(3)
            for q in range(NQ // 2):
                emit_w_quad(1, q)
            for mi in range(4, 8):
                emit_x_dma(mi)
            for q in range(NQ // 2, NQ):
                emit_w_quad(1, q)
            for q in range(NQ):
                emit_w_quad(2, q)
            for mi in range(8, 12):
                emit_x_dma(mi)
            for q in range(NQ):
                emit_w_quad(3, q)
            for mi in range(12, MT):
                emit_x_dma(mi)

            # PE warmup: bridge until w block 0 is signed and keep the HAM
            # clock gate moving to 2.4 GHz before the real stream starts.
            warm = wpp.tile([P, P], fp32, tag="warm", name="warm")
            for _ in range(n_warmup):
                nc.tensor.matmul(warm, ident, ident, start=True, stop=True)

            def emit_group(mi, bi):
                bsl = slice(bi * NT, (bi + 1) * NT)
                xbT = xbts[mi]
                mp = mpp.tile([P, NT], fp32, tag="mp", name=f"mp_{mi}_{bi}")
                for j2 in range(KSUB // 2):
                    q, r = divmod(j2, 2)
                    nc.tensor.matmul(
                        mp,
                        xbT[:, 2 * j2 : 2 * j2 + 2, :],
                        wq[bi * NQ + q][:, 2 * r : 2 * r + 2, :],
                        start=(j2 == 0),
                        stop=(j2 == KSUB // 2 - 1),
                        perf_mode=mybir.MatmulPerfMode.DoubleRow,
                    )
                ost = ostp.tile([P, NT], bf16, tag="ost", name=f"ost_{mi}_{bi}")
                nc.vector.tensor_tensor(
                    ost, mp, bias_rep[:, bsl], op=mybir.AluOpType.add
                )
                nc.gpsimd.dma_start(out[mi * P : (mi + 1) * P, bsl], ost)

            # PE order: per m-block, column-block sweeps. DVE binarizes are
            # threaded between group tails so the FIFO never heads-of-line
            # blocks a bias-add for long.
            for mb in range(MT // mblk_size):
                blk = list(range(mb * mblk_size, (mb + 1) * mblk_size))
                for bi in range(NB):
                    for k, mi in enumerate(blk):
                        emit_x_bin(mi)
                        emit_group(mi, bi)
                        if mb == 0 and bi == 0 and mi + 2 < mblk_size:
                            # binarize x{mi+2} right behind this group's
                            # bias-add (its DMA has just landed)
                            emit_x_bin(mi + 2)
                        if mb == 0 and bi >= 2 and k < 4:
                            # next m-block's tiles: x8-11 during the b2
                            # sweep, x12-15 during b3 (after their DMAs)
                            nxt = mblk_size + 4 * (bi - 2) + k
                            if nxt < MT:
                                emit_x_bin(nxt)
    nc.finalize()
    return nc


def permute_x(x_rows, K=K_FULL):
    """[M, K] fp32 -> [M//P, P, KSUB, P] bf16 per-m-tile [ki, j, m] lhsT."""
    M = x_rows.shape[0]
    ksub = K // P
    r = x_rows.reshape(M // P, P, ksub, P)  # [mi, m, j, ki]
    return np.ascontiguousarray(r.transpose(0, 3, 2, 1)).astype(BF16)


def permute_w(w_col, K=K_FULL, N=N_SHARD, quad=4, nt=NT):
    """[K, N] fp32 -> [NB*NQ, P, QUAD, NT] bf16 quad-major device layout."""
    nq = K // (P * quad)
    nb = N // nt
    r = w_col.reshape(nq, quad, P, nb, nt)
    return np.ascontiguousarray(
        r.transpose(3, 0, 2, 1, 4).reshape(nb * nq, P, quad, nt)
    ).astype(BF16)


def _make_in_maps(input, weight, bias):
    x_np = np.asarray(input, dtype=np.float32)
    w_np = np.asarray(weight, dtype=np.float32)
    b_np = np.asarray(bias, dtype=np.float32)
    x_rows = [
        permute_x(x_np[r * M_SHARD : (r + 1) * M_SHARD, :]) for r in range(R_SHARDS)
    ]
    w_cols = []
    b_cols = []
    for c in range(C_SHARDS):
        w_col = w_np[:, c * N_SHARD : (c + 1) * N_SHARD]
        w_cols.append(permute_w(w_col))
        # b' = bias - colsum(sign w): with x_hat = 2*(x>0) in {0,2},
        # sum_k sign(x) sign(w) = sum_k x_hat*sign(w) - sum_k sign(w).
        colsum = (2.0 * np.count_nonzero(w_col > 0, axis=0) - K_FULL).astype(
            np.float32
        )
        bp = (b_np[c * N_SHARD : (c + 1) * N_SHARD] - colsum).reshape(1, -1)
        b_cols.append(
            np.ascontiguousarray(np.broadcast_to(bp, (P, N_SHARD))).astype(np.float32)
        )
    in_maps = []
    for core in range(N_CORES):
        r, c = divmod(core, C_SHARDS)
        in_maps.append({"x": x_rows[r], "w": w_cols[c], "b": b_cols[c]})
    return in_maps


def _assemble(results):
    out = np.empty((M_FULL, N_FULL), dtype=np.float32)
    for core in range(N_CORES):
        r, c = divmod(core, C_SHARDS)
        out[r * M_SHARD : (r + 1) * M_SHARD, c * N_SHARD : (c + 1) * N_SHARD] = (
            results[core]["out"].astype(np.float32)
        )
    return out


def run(input, weight, bias, trace=False, trace_cores=None):
    """Run on 8 NeuronCores; returns (output, BassKernelResults)."""
    from concourse.bass_utils import run_bass_kernel_spmd

    nc = build_nc()
    in_maps = _make_in_maps(input, weight, bias)
    res = run_bass_kernel_spmd(
        nc, in_maps, list(range(N_CORES)), trace=trace, trace_cores=trace_cores
    )
    return _assemble(res.results), res


def kernel(input, weight, bias):
    out, _ = run(input, weight, bias)
    return out


# revision 10
# speedup vs baseline: 1.3253x; 1.0336x over previous
"""BinarizeLinear inference kernel for 8 Trainium2 NeuronCores.

Computes out = sign(input) @ sign(weight) + bias with sign(x) = +1 if x > 0
else -1, for input [8192, 4096] fp32, weight [4096, 4096] fp32, bias [4096].

Strategy: 4x2 (rows x cols) sharding across 8 cores — the DMA-optimal split.
Each core computes a [2048, 2048] output shard from x rows [2048, 4096] and
w cols [4096, 2048].

v3 design (vs the 342 us v2 baseline):
  - inputs staged to the device as bf16 (sign-exact for randn data): input
    DMA halves to 32 MiB/core, so the in-stream lands in ~80 us and the
    ramp is no longer DMA-starved
  - x is binarized on the DVE as x_hat = 2*(x>0) in {0,2} fp8 (one
    tensor_scalar op); the identity sum(sign x * sign w) =
    sum(x_hat * sign w) - colsum(sign w) is folded into a host-adjusted
    bias b' = bias - colsum(sign w), so ACT only signs w (64 us) and is
    never the ramp gate
  - main GEMM unchanged: fp8 DoubleRow, 256-deep per matmul, fp32 PSUM
    (exact: partial sums are even integers <= 8192), ~213 ns per
    [256 x 128] x [256 x 512] matmul warm -> 218 us PE floor per core
  - output written as bf16 (exact to ~1e-3 of max; tolerance is 2e-2),
    halving the out stream to 8 MiB
  - DMA / ACT / DVE streams hand-ordered so each w block and x tile is
    resident just before the in-order PE stream reaches it
"""

import numpy as np
import ml_dtypes

M_FULL, K_FULL, N_FULL = 8192, 4096, 4096
R_SHARDS, C_SHARDS = 4, 2
N_CORES = R_SHARDS * C_SHARDS
M_SHARD = M_FULL // R_SHARDS  # 2048
N_SHARD = N_FULL // C_SHARDS  # 2048
P = 128
NT = 512  # moving free dim per matmul (one PSUM bank of fp32)

BF16 = ml_dtypes.bfloat16


def build_nc(M=M_SHARD, K=K_FULL, N=N_SHARD, mblk_size=8, n_warmup=90):
    """Build the single-core Bass program (SPMD: same program on all cores)."""
    import concourse.mybir as mybir
    from concourse import bacc
    from concourse.tile import TileContext

    fp32 = mybir.dt.float32
    bf16 = mybir.dt.bfloat16
    fp8 = mybir.dt.float8e4

    QUAD = 4  # k-chunks per w tile
    assert M % P == 0 and K % (P * QUAD) == 0 and N % NT == 0
    KSUB = K // P  # 32 k-chunks of 128
    NQ = KSUB // QUAD  # 8 w quad tiles per n-block
    NB = N // NT  # 4 output column blocks
    MT = M // P  # 16 m-tiles
    mblk_size = min(mblk_size, MT)
    assert MT % mblk_size == 0 and KSUB % 2 == 0

    nc = bacc.Bacc()
    # x pre-permuted on host per m-tile: x_dev[mi, ki, j, m] = x[mi*P+m, j*P+ki]
    # (bf16) — each m-tile is one contiguous 1 MiB DMA in lhsT layout.
    x = nc.declare_dram_parameter("x", [MT, P, KSUB, P], bf16, isOutput=False)
    # w pre-permuted on host into quad-major layout (bf16):
    # w_dev[b*NQ+q, ki, j, n] = w[(q*QUAD+j)*P + ki, b*NT + n] — 0.5 MiB quads.
    w = nc.declare_dram_parameter("w", [NB * NQ, P, QUAD, NT], bf16, isOutput=False)
    # b' = bias - colsum(sign w), pre-replicated across partitions (fp32).
    b = nc.declare_dram_parameter("b", [P, N], fp32, isOutput=False)
    out = nc.declare_dram_parameter("out", [M, N], bf16, isOutput=True)

    with TileContext(nc) as tc:
        with (
            tc.tile_pool(name="const", bufs=1) as cpool,
            tc.tile_pool(name="win", bufs=8) as winp,
            tc.tile_pool(name="wsgn", bufs=2) as wsgnp,
            tc.tile_pool(name="wq", bufs=1) as wqp,
            tc.tile_pool(name="xin", bufs=3) as xinp,
            tc.tile_pool(name="xbt", bufs=1) as xbtp,
            tc.tile_pool(name="ost", bufs=6) as ostp,
            tc.tile_pool(name="mpsum", bufs=7, space="PSUM") as mpp,
            tc.tile_pool(name="wpsum", bufs=1, space="PSUM") as wpp,
        ):
            # Warmup stationary operand: memset fp8 zeros (no identity
            # needed — v3+ has no on-chip transposes).
            warm_lhs = cpool.tile([P, P], fp8)
            nc.vector.memset(warm_lhs, 0)

            bias_rep = cpool.tile([P, N], fp32)
            # gpsimd SWDGE: keeps the 1 MiB bias off the input-critical
            # HWDGE queues at t=0 (it isn't needed until ~35 us).
            nc.gpsimd.dma_start(bias_rep, b[:, :])

            wq = [None] * (NB * NQ)

            def emit_w_dma(bi, q):
                # w rides the scalar HWDGE queue: the trigger sits in the
                # ACT FIFO right before its own sign, so the stream is
                # self-paced and never blocks the x stream (sync queue).
                w_in = winp.tile([P, QUAD, NT], bf16, tag="w_in", name=f"w_in_{bi}_{q}")
                nc.scalar.dma_start(w_in, w[bi * NQ + q])
                return w_in

            def emit_w_sign_act(bi, q, w_in):
                wt = wqp.tile([P, QUAD, NT], fp8, tag=f"wq{bi}_{q}", name=f"wq_{bi}_{q}")
                nc.scalar.sign(wt, w_in)
                wq[bi * NQ + q] = wt

            def emit_w_sign_dve(bi, q, w_in):
                # sign via bf16 bit tricks on the DVE: (bits & 0x8000) |
                # 0x3F80 == bf16 +-1.0, then convert to fp8. Lets the DVE
                # and ACT sign block 0 concurrently during the ramp.
                wt16 = wsgnp.tile(
                    [P, QUAD, NT], bf16, tag="wsgn", name=f"wsgn_{bi}_{q}"
                )
                nc.vector.tensor_scalar(
                    wt16.bitcast(mybir.dt.uint16),
                    w_in.bitcast(mybir.dt.uint16),
                    0x8000,
                    0x3F80,
                    mybir.AluOpType.bitwise_and,
                    mybir.AluOpType.bitwise_or,
                )
                wt = wqp.tile([P, QUAD, NT], fp8, tag=f"wq{bi}_{q}", name=f"wq_{bi}_{q}")
                nc.vector.tensor_copy(wt, wt16)
                wq[bi * NQ + q] = wt

            xbts = [None] * MT
            x_binned = [False] * MT

            def emit_x_dma(mi):
                x_in = xinp.tile([P, KSUB, P], bf16, tag="x_in", name=f"x_in_{mi}")
                nc.sync.dma_start(x_in, x[mi])
                xbts[mi] = x_in  # raw tile until binarized

            def emit_x_bin(mi):
                if x_binned[mi]:
                    return
                x_binned[mi] = True
                x_in = xbts[mi]
                # 12 physical buffers: tile mi reuses mi-12's (dead by then)
                xbT = xbtp.tile(
                    [P, KSUB, P], fp8, tag=f"xbT{mi % 12}", name=f"xbT_{mi}"
                )
                # x_hat = 2 * (x > 0) in {0, 2}; exact in fp8.
                nc.vector.tensor_scalar(
                    xbT, x_in, 0.0, 2.0, mybir.AluOpType.is_gt, mybir.AluOpType.mult
                )
                xbts[mi] = xbT

            # ---- stream section ----
            # x tiles stream alone on the sync HWDGE queue in tile order;
            # w quads stream on the scalar queue paced by their signs.
            for mi in range(MT):
                emit_x_dma(mi)
            w_in_b0 = [emit_w_dma(0, q) for q in range(NQ)]
            # DVE: xb0, b0 signs for q0/q1, xb1, then q2/q3 (ACT does q4-7).
            emit_x_bin(0)
            emit_w_sign_dve(0, 0, w_in_b0[0])
            emit_w_sign_dve(0, 1, w_in_b0[1])
            emit_x_bin(1)
            emit_w_sign_dve(0, 2, w_in_b0[2])
            emit_w_sign_dve(0, 3, w_in_b0[3])
            emit_x_bin(2)
            emit_x_bin(3)
            for q in range(4, NQ):
                emit_w_sign_act(0, q, w_in_b0[q])
            for bi in range(1, NB):
                for q in range(NQ):
                    emit_w_sign_act(bi, q, emit_w_dma(bi, q))

            # PE warmup: bridge until w block 0 is signed and move the HAM
            # clock gate to 2.4 GHz before the real stream starts.
            warm = wpp.tile([P, P], fp32, tag="warm", name="warm")
            for _ in range(n_warmup):
                nc.tensor.matmul(warm, warm_lhs, warm_lhs, start=True, stop=True)

            def emit_group(mi, bi):
                bsl = slice(bi * NT, (bi + 1) * NT)
                xbT = xbts[mi]
                mp = mpp.tile([P, NT], fp32, tag="mp", name=f"mp_{mi}_{bi}")
                for j2 in range(KSUB // 2):
                    q, r = divmod(j2, 2)
                    nc.tensor.matmul(
                        mp,
                        xbT[:, 2 * j2 : 2 * j2 + 2, :],
                        wq[bi * NQ + q][:, 2 * r : 2 * r + 2, :],
                        start=(j2 == 0),
                        stop=(j2 == KSUB // 2 - 1),
                        perf_mode=mybir.MatmulPerfMode.DoubleRow,
                    )
                ost = ostp.tile([P, NT], bf16, tag="ost", name=f"ost_{mi}_{bi}")
                nc.vector.tensor_tensor(
                    ost, mp, bias_rep[:, bsl], op=mybir.AluOpType.add
                )
                if mi < mblk_size:
                    nc.gpsimd.dma_start(out[mi * P : (mi + 1) * P, bsl], ost)
                else:
                    # second m-block's stores ride the (long idle) sync
                    # HWDGE queue: at kernel end only a fast HWDGE drain
                    # remains, not the ~6 us SWDGE ring drain.
                    nc.sync.dma_start(out[mi * P : (mi + 1) * P, bsl], ost)

            # PE order: per m-block, column-block sweeps. DVE binarizes are
            # threaded between group tails so the FIFO never heads-of-line
            # blocks a bias-add for long.
            for mb in range(MT // mblk_size):
                blk = list(range(mb * mblk_size, (mb + 1) * mblk_size))
                for bi in range(NB):
                    for k, mi in enumerate(blk):
                        emit_x_bin(mi)
                        emit_group(mi, bi)
                        if mb == 0 and bi == 0 and mi + 2 < mblk_size:
                            # binarize x{mi+2} right behind this group's
                            # bias-add (its DMA has just landed)
                            emit_x_bin(mi + 2)
                        if mb == 0 and bi >= 2 and k < 4:
                            # next m-block's tiles: x8-11 during the b2
                            # sweep, x12-15 during b3 (after their DMAs)
                            nxt = mblk_size + 4 * (bi - 2) + k
                            if nxt < MT:
                                emit_x_bin(nxt)
    nc.finalize()
    return nc


def permute_x(x_rows, K=K_FULL):
    """[M, K] fp32 -> [M//P, P, KSUB, P] bf16 per-m-tile [ki, j, m] lhsT."""
    M = x_rows.shape[0]
    ksub = K // P
    r = x_rows.reshape(M // P, P, ksub, P)  # [mi, m, j, ki]
    return np.ascontiguousarray(r.transpose(0, 3, 2, 1)).astype(BF16)


def permute_w(w_col, K=K_FULL, N=N_SHARD, quad=4, nt=NT):
    """[K, N] fp32 -> [NB*NQ, P, QUAD, NT] bf16 quad-major device layout."""
    nq = K // (P * quad)
    nb = N // nt
    r = w_col.reshape(nq, quad, P, nb, nt)
    return np.ascontiguousarray(
        r.transpose(3, 0, 2, 1, 4).reshape(nb * nq, P, quad, nt)
    ).astype(BF16)


def _make_in_maps(input, weight, bias):
    x_np = np.asarray(input, dtype=np.float32)
    w_np = np.asarray(weight, dtype=np.float32)
    b_np = np.asarray(bias, dtype=np.float32)
    x_rows = [
        permute_x(x_np[r * M_SHARD : (r + 1) * M_SHARD, :]) for r in range(R_SHARDS)
    ]
    w_cols = []
    b_cols = []
    for c in range(C_SHARDS):
        w_col = w_np[:, c * N_SHARD : (c + 1) * N_SHARD]
        w_cols.append(permute_w(w_col))
        # b' = bias - colsum(sign w): with x_hat = 2*(x>0) in {0,2},
        # sum_k sign(x) sign(w) = sum_k x_hat*sign(w) - sum_k sign(w).
        colsum = (2.0 * np.count_nonzero(w_col > 0, axis=0) - K_FULL).astype(
            np.float32
        )
        bp = (b_np[c * N_SHARD : (c + 1) * N_SHARD] - colsum).reshape(1, -1)
        b_cols.append(
            np.ascontiguousarray(np.broadcast_to(bp, (P, N_SHARD))).astype(np.float32)
        )
    in_maps = []
    for core in range(N_CORES):
        r, c = divmod(core, C_SHARDS)
        in_maps.append({"x": x_rows[r], "w": w_cols[c], "b": b_cols[c]})
    return in_maps


def _assemble(results):
    out = np.empty((M_FULL, N_FULL), dtype=np.float32)
    for core in range(N_CORES):
        r, c = divmod(core, C_SHARDS)
        out[r * M_SHARD : (r + 1) * M_SHARD, c * N_SHARD : (c + 1) * N_SHARD] = (
            results[core]["out"].astype(np.float32)
        )
    return out


def run(input, weight, bias, trace=False, trace_cores=None):
    """Run on 8 NeuronCores; returns (output, BassKernelResults)."""
    from concourse.bass_utils import run_bass_kernel_spmd

    nc = build_nc()
    in_maps = _make_in_maps(input, weight, bias)
    res = run_bass_kernel_spmd(
        nc, in_maps, list(range(N_CORES)), trace=trace, trace_cores=trace_cores
    )
    return _assemble(res.results), res


def kernel(input, weight, bias):
    out, _ = run(input, weight, bias)
    return out
